# revision 38
# baseline (speedup 1.0000x reference)
"""Trainium2 Bass kernel for BilinearInteraction.

Reference math (B=2048, F=32 fields, D=64, P=496 field-pairs):
    for pair p=(i,j):  out[b,p,:] = (v_i @ W[p].T) * v_j
    v_i = feature_emb[:, i, :],  v_j = feature_emb[:, j, :]

Sharding: data-parallel over batch, 8 cores x 256 rows each; W replicated.
The fp32 output is 260MB (32.5MB/core) -> the kernel is output-write bound,
so the device writes bf16 (16.25MB/core) and the host upcasts.

Per-core dataflow (all static, Tile-scheduled):
  - wtile[128, 16384] bf16 resident: partitions 0:64 hold pairs 0..255
    (col p*64+e = W[p,e,d=partition]), partitions 64:128 pairs 256..495.
    Loaded as column-sliced DMAs in demand order.
  - ft[128, 5632] bf16: per-field transposed features (matmul lhsT).
    Partitions 0:64 = fields 0..9 (col f*256+b), 64:128 = fields 9..30.
  - fn32[2][128, 2048] f32: natural-layout features (Hadamard
    multiplier); a bf16 copy fnb is derived on-chip (DVE 2x_2p copy).
  - Pairs processed in field order; a "run" = <=16 consecutive pairs with
    the same first field and half (one 1-2 bank PSUM tile, 1-2 matmuls).
    The PSUM x v_j Hadamard is routed per-run to one of three engine
    paths, balancing measured engine rates (e/ns: DVE-f32-psum 104,
    DVE-bf16-2x 266, ACT-copy-to-bf16 101, ACT-copy-to-f32 135, GPS-f32
    75; GPS-bf16 is slower than f32 so path C stays f32):
      A: DVE  tensor_mul(psum f32, fn32)    -> st bf16
      D: ACT  copy psum -> tmpD bf16; DVE mul(tmpD, fnb)   (2x mode)
      C: ACT  copy psum -> tmpC f32;  GPS mul(tmpC, fn32)
    This keeps DVE/ACT/GPS each at ~40us instead of DVE+GPS at ~52us.
  - Output stages of <=64 consecutive pairs buffer TT results in st bf16
    tiles; one DMA per stage. Input DMAs ride the scalar HWDGE ring in
    compute-demand order; output DMAs alternate sync/scalar rings, with
    the scalar-ring ones deferred by one stage so ACT copies never stall
    behind an output DMA's semaphore wait.
"""

from itertools import combinations

import numpy as np

N_CORES = 8
B, F, D = 2048, 32, 64
P = 496
B_SH = B // N_CORES            # 256 batch rows per core
HALF = 256                     # pair index where the partition half flips
RUN = 16                       # max pairs per Hadamard op (2 PSUM banks)

PAIRS = list(combinations(range(F), 2))

_NC_CACHE = {}


def _runs(half):
    """Runs of consecutive same-field pairs (<=RUN) within one partition
    half. Returns [(p0, n), ...] in pair order."""
    lo, hi = (0, HALF) if half == 0 else (HALF, P)
    runs = []
    p = 0
    for i in range(F - 1):
        size = F - 1 - i
        s0, s1 = max(p, lo), min(p + size, hi)
        q = s0
        while q < s1:
            n = min(RUN, s1 - q)
            runs.append((q, n))
            q += n
        p += size
    return runs


def _stages(runs, caps):
    """Greedy-pack consecutive runs into output stages (<= cap pairs)."""
    stages, cur, cnt, ci = [], [], 0, 0
    for r in runs:
        cap = caps[min(ci, len(caps) - 1)]
        if cur and cnt + r[1] > cap:
            stages.append(cur)
            ci += 1
            cur, cnt = [], 0
        cur.append(r)
        cnt += r[1]
    if cur:
        stages.append(cur)
    return stages


class _Balance:
    """Deficit-based per-run engine-path assignment toward element-count
    targets that equalize DVE/ACT/GPS busy time (A: DVE 1x from PSUM,
    D: ACT copy + DVE 2x, C: ACT copy + GPS)."""

    TARGET = {"A": 0.654, "D": 0.0, "C": 0.346}

    def __init__(self):
        self.el = {"A": 0.0, "D": 0.0, "C": 0.0}
        self.tot = 0.0

    def __init_streak(self):
        pass

    def pick(self, n, allow_c=True):
        if n <= 4:
            path = "A"          # tiny runs: per-op overhead cheapest on DVE
        else:
            cands = "AC" if (n >= 10 and allow_c) else "A"
            if len(cands) > 1 and getattr(self, "streak", ("", 0))[1] >= 2:
                cands = cands.replace(self.streak[0], "") or cands
            path = max(cands,
                       key=lambda q: self.TARGET[q] * self.tot - self.el[q])
        s = getattr(self, "streak", ("", 0))
        self.streak = (path, s[1] + 1 if s[0] == path else 1)
        self.el[path] += n
        self.tot += n
        return path


def _build():
    import concourse.tile as tile
    from concourse import bacc, mybir

    F32 = mybir.dt.float32
    BF16 = mybir.dt.bfloat16
    nc = bacc.Bacc("TRN2", target_bir_lowering=False, debug=False,
                   enable_asserts=False, num_devices=N_CORES)

    wpack = nc.dram_tensor("wpack", [128, 16384], BF16, kind="ExternalInput").ap()
    featT = nc.dram_tensor("featT", [128, 22 * B_SH], BF16, kind="ExternalInput").ap()
    featN = nc.dram_tensor("featN", [B_SH, F * D], BF16, kind="ExternalInput").ap()
    out = nc.dram_tensor("out", [B_SH, P * D], BF16, kind="ExternalOutput").ap()

    # stages alternate between the two partition halves: pairs p and
    # HALF+p share the same w columns, so interleaving halves the rate at
    # which the PE walks into not-yet-DMAed w data during the input phase.
    def interleaved(c0, c1):
        s_h0 = _stages(_runs(0), caps=c0)
        s_h1 = _stages(_runs(1), caps=c1)
        out_s = []
        for a, b in zip(s_h0, s_h1):
            out_s += [a, b]
        n = min(len(s_h0), len(s_h1))
        out_s += s_h0[n:] + s_h1[n:]
        return out_s

    # uniform stages: the small-first ramp predates the sync-ring output
    # path and only added stage boundaries; output start time is no longer
    # critical with 10 stage buffers
    stages0 = interleaved([64] * 16, [64] * 16)
    stages1 = interleaved([64] * 16, [64] * 16)
    bal = _Balance()

    with tile.TileContext(nc) as tc:
        with (
            tc.tile_pool(name="win", bufs=1) as win,
            tc.tile_pool(name="feat", bufs=1) as feat,
            tc.tile_pool(name="stage", bufs=10) as stage_pool,
            tc.tile_pool(name="tmp", bufs=8) as tmp_pool,
            tc.tile_pool(name="psum", bufs=4, space="PSUM") as psum_pool,
        ):
            # resident input tiles ------------------------------------------------
            w = win.tile([128, 16384], BF16, name="w", tag="w")
            ft = feat.tile([128, 22 * B_SH], BF16, name="ft", tag="ft")
            fnb = [feat.tile([128, F * D], BF16, name=f"fnb{bc}", tag=f"fnb{bc}")
                   for bc in range(2)]

            # input DMAs ride the SYNC HWDGE ring (which does nothing
            # else), striped in compute-demand order: w cols feed pairs at
            # ~64 pairs/512KB, ft cols feed fields. Keeping them off the
            # scalar ring matters: a dma_start occupies its engine's
            # sequencer until the ring accepts the descriptors, so 7.4MB of
            # input descriptors on the ACT queue would block the first ACT
            # copy until ~23us (observed in earlier revisions).
            in_slices = [
                (ft[:, 0:256], featT[:, 0:256]),
                (w[:, 0:1024], wpack[:, 0:1024]),
                (ft[:, 256:512], featT[:, 256:512]),
                (w[:, 1024:2048], wpack[:, 1024:2048]),
                (fnb[0][:, :], featN[0:128, :]),
                (w[:, 2048:4096], wpack[:, 2048:4096]),
                (fnb[1][:, :], featN[128:256, :]),
                (ft[:, 512:1536], featT[:, 512:1536]),
                (w[:, 4096:6144], wpack[:, 4096:6144]),
                (ft[:, 1536:2560], featT[:, 1536:2560]),
                (w[:, 6144:8192], wpack[:, 6144:8192]),
                (ft[:, 2560:3584], featT[:, 2560:3584]),
                (w[:, 8192:10240], wpack[:, 8192:10240]),
                (ft[:, 3584:5632], featT[:, 3584:5632]),
                (w[:, 10240:12288], wpack[:, 10240:12288]),
                (w[:, 12288:14336], wpack[:, 12288:14336]),
                (w[:, 14336:16384], wpack[:, 14336:16384]),
            ]
            # prime the pipe with the first slices; the tail is issued one
            # slice per output stage, interleaved on the same sync ring so
            # inputs always precede the just-finished stage's output in the
            # ring FIFO (a compute engine never issues a DMA: the convoy
            # where an output DMA's sem-wait blocked the ACT copy queue was
            # worth ~7us per occurrence)
            for dst, s_ in in_slices[:7]:
                nc.sync.dma_start(dst, s_)
            in_next = [7]

            # compute + output ----------------------------------------------------

            # derive the f32 multiplier copies for paths A/C on-chip (the
            # bf16 original is what gets DMAed: halves input bytes); on ACT,
            # which has slack -- DVE is the scarce engine. fn32[1] is issued
            # after bc0's first stage so it lands well before bc1 starts but
            # does not gate bc0's pipe startup.
            for bc in range(2):
                stages = stages0 if bc == 0 else stages1
                n_st = len(stages)
                for si, stage in enumerate(stages):
                    lo = stage[0][0]
                    hi = stage[-1][0] + stage[-1][1]
                    st = stage_pool.tile([128, (hi - lo) * D], BF16, tag="stage")
                    for p0, n in stage:
                        i, j0 = PAIRS[p0]
                        h = p0 // HALF
                        po = 64 * h
                        fcol = (i - 9 * h) * B_SH + bc * 128
                        colbase = (p0 - h * HALF) * D
                        ps = psum_pool.tile([128, RUN * D], F32, tag="ps",
                                            bufs=4)
                        for k in range(0, n, 8):
                            nk = min(8, n - k)
                            nc.tensor.matmul(
                                ps[:, k * D:(k + nk) * D],
                                lhsT=ft[po:po + 64, fcol:fcol + 128],
                                rhs=w[po:po + 64,
                                      colbase + k * D: colbase + (k + nk) * D],
                                start=True, stop=True,
                            )
                        st_sl = st[:, (p0 - lo) * D: (p0 - lo + n) * D]
                        fnb_sl = fnb[bc][:, j0 * D: (j0 + n) * D]
                        path = bal.pick(n, allow_c=(bc == 0 or si < n_st - 1))
                        if path == "A":
                            nc.vector.tensor_mul(st_sl, ps[:, 0:n * D], fnb_sl)
                        elif path == "D":
                            tmp = tmp_pool.tile([128, RUN * D], BF16, tag="tmpD",
                                                bufs=4)
                            fnb_sl = fnb[bc][:, j0 * D: (j0 + n) * D]
                            nc.scalar.copy(tmp[:, 0:n * D], ps[:, 0:n * D])
                            nc.vector.tensor_mul(st_sl, tmp[:, 0:n * D], fnb_sl)
                        else:
                            tmp = tmp_pool.tile([128, RUN * D], F32, tag="tmpC",
                                                bufs=6)
                            nc.scalar.copy(tmp[:, 0:n * D], ps[:, 0:n * D])
                            nc.gpsimd.tensor_mul(st_sl, tmp[:, 0:n * D], fnb_sl)
                    # output DMA: sync ring immediately, or scalar ring
                    # deferred one stage (avoids ACT head-of-line stalls)
                    for _ in range(2):
                        if in_next[0] < len(in_slices):
                            dst, s_ = in_slices[in_next[0]]
                            nc.sync.dma_start(dst, s_)
                            in_next[0] += 1
                    nc.sync.dma_start(out[bc * 128: bc * 128 + 128,
                                          lo * D: hi * D], st[:, :])

    nc.compile()
    return nc


def _pack_inputs(feature_emb, W):
    import ml_dtypes

    BF = ml_dtypes.bfloat16
    feature_emb = np.ascontiguousarray(feature_emb, dtype=np.float32)
    W = np.ascontiguousarray(W, dtype=np.float32)
    Wt = W.transpose(0, 2, 1)                      # [P, d, e]
    wpack = np.zeros((128, 16384), dtype=BF)
    wpack[0:64, :] = Wt[0:HALF].transpose(1, 0, 2).reshape(64, HALF * D).astype(BF)
    wpack[64:128, 0:(P - HALF) * D] = (
        Wt[HALF:P].transpose(1, 0, 2).reshape(64, (P - HALF) * D).astype(BF))
    in_maps = []
    for c in range(N_CORES):
        shard = feature_emb[c * B_SH:(c + 1) * B_SH]         # [256, 32, 64]
        # [d, f, b] per-field transposed features
        ftT = shard.transpose(2, 1, 0).astype(BF)            # [64, 32, 256]
        featT = np.zeros((128, 22 * B_SH), dtype=BF)
        # partitions 0:64 <- fields 0..9 (first-fields of pairs 0..255)
        featT[0:64, 0:10 * B_SH] = ftT[:, 0:10].reshape(64, 10 * B_SH)
        # partitions 64:128 <- fields 9..30 (first-fields of pairs 256..495)
        featT[64:128, :] = ftT[:, 9:31].reshape(64, 22 * B_SH)
        in_maps.append({
            "wpack": wpack,
            "featT": featT,
            "featN": np.ascontiguousarray(shard.reshape(B_SH, F * D)).astype(BF),
        })
    return in_maps


def kernel(feature_emb, W, _trace=False):
    from concourse.bass_utils import run_bass_kernel_spmd

    if "nc" not in _NC_CACHE:
        _NC_CACHE["nc"] = _build()
    nc = _NC_CACHE["nc"]
    in_maps = _pack_inputs(feature_emb, W)
    res = run_bass_kernel_spmd(nc, in_maps, core_ids=list(range(N_CORES)),
                               trace=_trace)
    full = np.concatenate(
        [res.results[c]["out"].astype(np.float32) for c in range(N_CORES)], axis=0)
    out = full.reshape(B, P, D)
    if _trace:
        return out, res
    return out


# revision 40
# speedup vs baseline: 1.1043x; 1.1043x over previous
"""Trainium2 Bass kernel for BilinearInteraction.

Reference math (B=2048, F=32 fields, D=64, P=496 field-pairs):
    for pair p=(i,j):  out[b,p,:] = (v_i @ W[p].T) * v_j
    v_i = feature_emb[:, i, :],  v_j = feature_emb[:, j, :]

Sharding: data-parallel over batch, 8 cores x 256 rows each; W replicated.
The fp32 output is 260MB (32.5MB/core) -> the kernel is output-write bound,
so the device writes bf16 (16.25MB/core) and the host upcasts.

Per-core dataflow (all static, Tile-scheduled):
  - wtile[128, 16384] bf16 resident: partitions 0:64 hold pairs 0..255
    (col p*64+e = W[p,e,d=partition]), partitions 64:128 pairs 256..495.
    Loaded as column-sliced DMAs in demand order.
  - ft[128, 5632] bf16: per-field transposed features (matmul lhsT).
    Partitions 0:64 = fields 0..9 (col f*256+b), 64:128 = fields 9..30.
  - fn32[2][128, 2048] f32: natural-layout features (Hadamard
    multiplier); a bf16 copy fnb is derived on-chip (DVE 2x_2p copy).
  - Pairs processed in field order; a "run" = <=16 consecutive pairs with
    the same first field and half (one 1-2 bank PSUM tile, 1-2 matmuls).
    The PSUM x v_j Hadamard is routed per-run to one of three engine
    paths, balancing measured engine rates (e/ns: DVE-f32-psum 104,
    DVE-bf16-2x 266, ACT-copy-to-bf16 101, ACT-copy-to-f32 135, GPS-f32
    75; GPS-bf16 is slower than f32 so path C stays f32):
      A: DVE  tensor_mul(psum f32, fn32)    -> st bf16
      D: ACT  copy psum -> tmpD bf16; DVE mul(tmpD, fnb)   (2x mode)
      C: ACT  copy psum -> tmpC f32;  GPS mul(tmpC, fn32)
    This keeps DVE/ACT/GPS each at ~40us instead of DVE+GPS at ~52us.
  - Output stages of <=64 consecutive pairs buffer TT results in st bf16
    tiles; one DMA per stage. Input DMAs ride the scalar HWDGE ring in
    compute-demand order; output DMAs alternate sync/scalar rings, with
    the scalar-ring ones deferred by one stage so ACT copies never stall
    behind an output DMA's semaphore wait.
"""

from itertools import combinations

import numpy as np

N_CORES = 8
B, F, D = 2048, 32, 64
P = 496
B_SH = B // N_CORES            # 256 batch rows per core
HALF = 256                     # pair index where the partition half flips
RUN = 16                       # max pairs per Hadamard op (2 PSUM banks)

PAIRS = list(combinations(range(F), 2))

_NC_CACHE = {}


def _runs(half):
    """Runs of consecutive same-field pairs (<=RUN) within one partition
    half. Returns [(p0, n), ...] in pair order."""
    lo, hi = (0, HALF) if half == 0 else (HALF, P)
    runs = []
    p = 0
    for i in range(F - 1):
        size = F - 1 - i
        s0, s1 = max(p, lo), min(p + size, hi)
        q = s0
        while q < s1:
            n = min(RUN, s1 - q)
            runs.append((q, n))
            q += n
        p += size
    return runs


def _stages(runs, caps):
    """Greedy-pack consecutive runs into output stages (<= cap pairs)."""
    stages, cur, cnt, ci = [], [], 0, 0
    for r in runs:
        cap = caps[min(ci, len(caps) - 1)]
        if cur and cnt + r[1] > cap:
            stages.append(cur)
            ci += 1
            cur, cnt = [], 0
        cur.append(r)
        cnt += r[1]
    if cur:
        stages.append(cur)
    return stages


class _Balance:
    """Deficit-based per-run engine-path assignment toward element-count
    targets that equalize DVE/ACT/GPS busy time (A: DVE 1x from PSUM,
    D: ACT copy + DVE 2x, C: ACT copy + GPS)."""

    TARGET = {"A": 0.654, "D": 0.0, "C": 0.346}

    def __init__(self):
        self.el = {"A": 0.0, "D": 0.0, "C": 0.0}
        self.tot = 0.0

    def __init_streak(self):
        pass

    def pick(self, n, allow_c=True):
        if n <= 4:
            path = "A"          # tiny runs: per-op overhead cheapest on DVE
        else:
            cands = "AC" if (n >= 10 and allow_c) else "A"
            if len(cands) > 1 and getattr(self, "streak", ("", 0))[1] >= 2:
                cands = cands.replace(self.streak[0], "") or cands
            path = max(cands,
                       key=lambda q: self.TARGET[q] * self.tot - self.el[q])
        s = getattr(self, "streak", ("", 0))
        self.streak = (path, s[1] + 1 if s[0] == path else 1)
        self.el[path] += n
        self.tot += n
        return path


def _build():
    import concourse.tile as tile
    from concourse import bacc, mybir

    F32 = mybir.dt.float32
    BF16 = mybir.dt.bfloat16
    nc = bacc.Bacc("TRN2", target_bir_lowering=False, debug=False,
                   enable_asserts=False, num_devices=N_CORES)

    wpack = nc.dram_tensor("wpack", [128, 16384], BF16, kind="ExternalInput").ap()
    featT = nc.dram_tensor("featT", [128, 22 * B_SH], BF16, kind="ExternalInput").ap()
    featN = nc.dram_tensor("featN", [B_SH, F * D], BF16, kind="ExternalInput").ap()
    out = nc.dram_tensor("out", [B_SH, P * D], BF16, kind="ExternalOutput").ap()

    # stages alternate between the two partition halves: pairs p and
    # HALF+p share the same w columns, so interleaving halves the rate at
    # which the PE walks into not-yet-DMAed w data during the input phase.
    def interleaved(c0, c1):
        s_h0 = _stages(_runs(0), caps=c0)
        s_h1 = _stages(_runs(1), caps=c1)
        out_s = []
        for a, b in zip(s_h0, s_h1):
            out_s += [a, b]
        n = min(len(s_h0), len(s_h1))
        out_s += s_h0[n:] + s_h1[n:]
        return out_s

    stages0 = interleaved([16, 32, 48] + [64] * 16, [32, 48] + [64] * 16)
    stages1 = interleaved([64] * 16, [64] * 16)
    bal = _Balance()

    with tile.TileContext(nc) as tc:
        with (
            tc.tile_pool(name="win", bufs=1) as win,
            tc.tile_pool(name="feat", bufs=1) as feat,
            tc.tile_pool(name="stage", bufs=10) as stage_pool,
            tc.tile_pool(name="tmp", bufs=8) as tmp_pool,
            tc.tile_pool(name="psum", bufs=4, space="PSUM") as psum_pool,
        ):
            # resident input tiles ------------------------------------------------
            w = win.tile([128, 16384], BF16, name="w", tag="w")
            ft = feat.tile([128, 22 * B_SH], BF16, name="ft", tag="ft")
            fnb = [feat.tile([128, F * D], BF16, name=f"fnb{bc}", tag=f"fnb{bc}")
                   for bc in range(2)]

            # input DMAs ride the SYNC HWDGE ring (which does nothing
            # else), striped in compute-demand order: w cols feed pairs at
            # ~64 pairs/512KB, ft cols feed fields. Keeping them off the
            # scalar ring matters: a dma_start occupies its engine's
            # sequencer until the ring accepts the descriptors, so 7.4MB of
            # input descriptors on the ACT queue would block the first ACT
            # copy until ~23us (observed in earlier revisions).
            in_slices = [
                (ft[:, 0:256], featT[:, 0:256]),
                (w[:, 0:1024], wpack[:, 0:1024]),
                (ft[:, 256:512], featT[:, 256:512]),
                (w[:, 1024:2048], wpack[:, 1024:2048]),
                (fnb[0][:, :], featN[0:128, :]),
                (w[:, 2048:4096], wpack[:, 2048:4096]),
                (fnb[1][:, :], featN[128:256, :]),
                (ft[:, 512:1536], featT[:, 512:1536]),
                (w[:, 4096:6144], wpack[:, 4096:6144]),
                (ft[:, 1536:2560], featT[:, 1536:2560]),
                (w[:, 6144:8192], wpack[:, 6144:8192]),
                (ft[:, 2560:3584], featT[:, 2560:3584]),
                (w[:, 8192:10240], wpack[:, 8192:10240]),
                (ft[:, 3584:5632], featT[:, 3584:5632]),
                (w[:, 10240:12288], wpack[:, 10240:12288]),
                (w[:, 12288:14336], wpack[:, 12288:14336]),
                (w[:, 14336:16384], wpack[:, 14336:16384]),
            ]
            # prime the pipe with the first slices; the tail is issued one
            # slice per output stage, interleaved on the same sync ring so
            # inputs always precede the just-finished stage's output in the
            # ring FIFO (a compute engine never issues a DMA: the convoy
            # where an output DMA's sem-wait blocked the ACT copy queue was
            # worth ~7us per occurrence)
            for dst, s_ in in_slices[:7]:
                nc.sync.dma_start(dst, s_)
            in_next = [7]

            # compute + output ----------------------------------------------------

            # derive the f32 multiplier copies for paths A/C on-chip (the
            # bf16 original is what gets DMAed: halves input bytes); on ACT,
            # which has slack -- DVE is the scarce engine. fn32[1] is issued
            # after bc0's first stage so it lands well before bc1 starts but
            # does not gate bc0's pipe startup.
            for bc in range(2):
                stages = stages0 if bc == 0 else stages1
                n_st = len(stages)
                for si, stage in enumerate(stages):
                    lo = stage[0][0]
                    hi = stage[-1][0] + stage[-1][1]
                    st = stage_pool.tile([128, (hi - lo) * D], BF16, tag="stage")
                    # very last stage: drain its first half early so the
                    # final DMA is half-size (cuts the post-compute tail)
                    last = (bc == 1 and si == n_st - 1)
                    half_done = False
                    for p0, n in stage:
                        if last and not half_done and p0 - lo >= (hi - lo) // 2:
                            nc.sync.dma_start(
                                out[bc * 128: bc * 128 + 128,
                                    lo * D: p0 * D],
                                st[:, 0:(p0 - lo) * D])
                            half_done = True
                            lo2 = p0
                        i, j0 = PAIRS[p0]
                        h = p0 // HALF
                        po = 64 * h
                        fcol = (i - 9 * h) * B_SH + bc * 128
                        colbase = (p0 - h * HALF) * D
                        ps = psum_pool.tile([128, RUN * D], F32, tag="ps",
                                            bufs=4)
                        for k in range(0, n, 8):
                            nk = min(8, n - k)
                            nc.tensor.matmul(
                                ps[:, k * D:(k + nk) * D],
                                lhsT=ft[po:po + 64, fcol:fcol + 128],
                                rhs=w[po:po + 64,
                                      colbase + k * D: colbase + (k + nk) * D],
                                start=True, stop=True,
                            )
                        st_sl = st[:, (p0 - lo) * D: (p0 - lo + n) * D]
                        fnb_sl = fnb[bc][:, j0 * D: (j0 + n) * D]
                        path = bal.pick(n, allow_c=(bc == 0 or si < n_st - 1))
                        if path == "A":
                            nc.vector.tensor_mul(st_sl, ps[:, 0:n * D], fnb_sl)
                        elif path == "D":
                            tmp = tmp_pool.tile([128, RUN * D], BF16, tag="tmpD",
                                                bufs=4)
                            fnb_sl = fnb[bc][:, j0 * D: (j0 + n) * D]
                            nc.scalar.copy(tmp[:, 0:n * D], ps[:, 0:n * D])
                            nc.vector.tensor_mul(st_sl, tmp[:, 0:n * D], fnb_sl)
                        else:
                            tmp = tmp_pool.tile([128, RUN * D], F32, tag="tmpC",
                                                bufs=6)
                            nc.scalar.copy(tmp[:, 0:n * D], ps[:, 0:n * D])
                            nc.gpsimd.tensor_mul(st_sl, tmp[:, 0:n * D], fnb_sl)
                    # output DMA: sync ring immediately, or scalar ring
                    # deferred one stage (avoids ACT head-of-line stalls)
                    for _ in range(2):
                        if in_next[0] < len(in_slices):
                            dst, s_ = in_slices[in_next[0]]
                            nc.sync.dma_start(dst, s_)
                            in_next[0] += 1
                    if last and half_done:
                        nc.sync.dma_start(
                            out[bc * 128: bc * 128 + 128, lo2 * D: hi * D],
                            st[:, (lo2 - lo) * D:])
                    else:
                        nc.sync.dma_start(out[bc * 128: bc * 128 + 128,
                                              lo * D: hi * D], st[:, :])

    nc.compile()
    return nc


def _pack_inputs(feature_emb, W):
    import ml_dtypes

    BF = ml_dtypes.bfloat16
    feature_emb = np.ascontiguousarray(feature_emb, dtype=np.float32)
    W = np.ascontiguousarray(W, dtype=np.float32)
    Wt = W.transpose(0, 2, 1)                      # [P, d, e]
    wpack = np.zeros((128, 16384), dtype=BF)
    wpack[0:64, :] = Wt[0:HALF].transpose(1, 0, 2).reshape(64, HALF * D).astype(BF)
    wpack[64:128, 0:(P - HALF) * D] = (
        Wt[HALF:P].transpose(1, 0, 2).reshape(64, (P - HALF) * D).astype(BF))
    in_maps = []
    for c in range(N_CORES):
        shard = feature_emb[c * B_SH:(c + 1) * B_SH]         # [256, 32, 64]
        # [d, f, b] per-field transposed features
        ftT = shard.transpose(2, 1, 0).astype(BF)            # [64, 32, 256]
        featT = np.zeros((128, 22 * B_SH), dtype=BF)
        # partitions 0:64 <- fields 0..9 (first-fields of pairs 0..255)
        featT[0:64, 0:10 * B_SH] = ftT[:, 0:10].reshape(64, 10 * B_SH)
        # partitions 64:128 <- fields 9..30 (first-fields of pairs 256..495)
        featT[64:128, :] = ftT[:, 9:31].reshape(64, 22 * B_SH)
        in_maps.append({
            "wpack": wpack,
            "featT": featT,
            "featN": np.ascontiguousarray(shard.reshape(B_SH, F * D)).astype(BF),
        })
    return in_maps


def kernel(feature_emb, W, _trace=False):
    from concourse.bass_utils import run_bass_kernel_spmd

    if "nc" not in _NC_CACHE:
        _NC_CACHE["nc"] = _build()
    nc = _NC_CACHE["nc"]
    in_maps = _pack_inputs(feature_emb, W)
    res = run_bass_kernel_spmd(nc, in_maps, core_ids=list(range(N_CORES)),
                               trace=_trace)
    full = np.concatenate(
        [res.results[c]["out"].astype(np.float32) for c in range(N_CORES)], axis=0)
    out = full.reshape(B, P, D)
    if _trace:
        return out, res
    return out
